# revision 1
# baseline (speedup 1.0000x reference)
"""Trainium2 Bass kernel for nn_DepthAttention (depth attention over d=32).

Reference computation (per pixel (b,h,w), all 1x1 convs):
  q = Wq x               [320]   (heads=8 x dh=40)
  k = Wk ctx[:, d]       [320, 32]
  v = Wv ctx[:, d]       [320, 32]
  sim[n,d] = sum_{c in head n} q[c] k[c,d] * scale
  attn = softmax_d(sim)
  o[c] = sum_d v[c,d] attn[head(c),d]
  y = Wout o + bout      [320]

Sharding: h (64) split across 8 cores -> 8 rows of h per core, no halo.
Per core: 1024 pixels in 8 blocks of P=128. The host pre-casts
context/x to bf16 and lays them out as per-block contiguous panels
(one 8 KiB DMA run per channel row). The context panel carries a 321st
constant-ones channel: the v-projection's chunk-2 matmul uses it to
emit softmax-denominator rows for free.

The block loop is software-pipelined depth 2: phase A (DMA, q/k
projections, k*q, selector-reduce to sim[8 rows]) of block i+1 is
emitted before phase B (broadcast+exp, v projection, v*attn, reduce,
normalize, output projection) of block i, so each engine's in-order
stream always has independent work while the other phase's
cross-engine chain settles.

Engine mapping:
  PE : all matmuls in bf16 (1 cyc/row, FWL weight loads): q/k/v
       projections, 320->8 selector reduce of k*q, 8->320 broadcast of
       sim, 1/den broadcast, Wout.
  DVE: k*q (q broadcast over d via step-0 AP), v*attn, reduce over d
       (d innermost via pixel-major v layout), reciprocal, bias add.
  ACT: psum->sbuf drains; exp AFTER broadcast on 128-row tiles.
       Softmax max-subtraction is skipped (logits are O(1) here).
"""

import sys

sys.path.insert(0, "/opt/trn_rl_repo")

from contextlib import ExitStack  # noqa: E402

import ml_dtypes  # noqa: E402
import numpy as np  # noqa: E402

import concourse.bacc as bacc  # noqa: E402
import concourse.bass as bass  # noqa: E402
import concourse.mybir as mybir  # noqa: E402
import concourse.tile as tile  # noqa: E402

HEADS = 8
DH = 40
CIN = 320
INNER = HEADS * DH  # 320
D = 32
B = 2
H = 64
W = 64
NCORES = 8
HLOC = H // NCORES  # 8
PIX_B = HLOC * W  # 512
P = 128
NBLK = B * PIX_B // P  # 8
NT = (D * P) // 512  # 8
SCALE = DH ** -0.5

F32 = mybir.dt.float32
F32R = mybir.dt.float32r
BF16 = mybir.dt.bfloat16
NPBF = ml_dtypes.bfloat16

CHUNKS = [(0, 128), (128, 128), (256, 64)]
# v-projection output sizes: chunk2 carries 8 extra denominator rows
VSZ = [128, 128, 72]


def _head_of(c):
    return c // DH


def make_constants():
    sel = np.zeros((128, 24), NPBF)
    for kc, (c0, csz) in enumerate(CHUNKS):
        for r in range(csz):
            sel[r, kc * 8 + _head_of(c0 + r)] = 1.0
    bsel = np.zeros((8, 384), NPBF)
    for mo, (c0, csz) in enumerate(CHUNKS):
        for r in range(csz):
            bsel[_head_of(c0 + r), mo * 128 + r] = 1.0
    for n in range(8):
        bsel[n, 2 * 128 + 64 + n] = 1.0
    rsel = np.zeros((8, 384), np.float32)
    for mo, (c0, csz) in enumerate(CHUNKS):
        for r in range(csz):
            rsel[_head_of(c0 + r), mo * 128 + r] = 1.0
    return sel, bsel, rsel


def pack_weight_T(w, ones_cols=False):
    """w [out, in] -> bf16 packed lhsT [128, 3*M] with M = out (+8 den
    cols when ones_cols).  Chunk kc of the 'in' dim at free offset kc*M;
    chunk 2 gets an extra contraction row 64 (the ones-channel), wired to
    the 8 denominator columns when ones_cols."""
    wt = np.ascontiguousarray(w.T, dtype=np.float32)  # [in, out]
    od = wt.shape[1]
    m = od + 8 if ones_cols else od
    p = np.zeros((128, 3 * m), NPBF)
    for kc, (c0, csz) in enumerate(CHUNKS):
        p[0:csz, kc * m:kc * m + od] = wt[c0:c0 + csz, :]
    if ones_cols:
        for n in range(8):
            p[64, 2 * m + od + n] = 1.0  # ones-channel -> den col n (chunk2)
    return p


def build_nc():
    nc = bacc.Bacc(
        "TRN2",
        target_bir_lowering=False,
        debug=False,
        enable_asserts=False,
        num_devices=NCORES,
    )

    ctx_t = nc.dram_tensor("ctx", [NBLK, CIN + 1, D * P], BF16, kind="ExternalInput")
    x_t = nc.dram_tensor("x", [NBLK, CIN, P], BF16, kind="ExternalInput")
    wq_t = nc.dram_tensor("wq_p", [128, 960], BF16, kind="ExternalInput")
    wk_t = nc.dram_tensor("wk_p", [128, 960], BF16, kind="ExternalInput")
    wv_t = nc.dram_tensor("wv_p", [128, 984], BF16, kind="ExternalInput")
    wo_t = nc.dram_tensor("wo_p", [128, 960], BF16, kind="ExternalInput")
    sel_t = nc.dram_tensor("sel_p", [128, 24], BF16, kind="ExternalInput")
    bsel_t = nc.dram_tensor("bsel_p", [8, 384], BF16, kind="ExternalInput")
    rsel_t = nc.dram_tensor("rsel_p", [8, 384], F32R, kind="ExternalInput")
    bout_t = nc.dram_tensor("bout_p", [128, 3], F32, kind="ExternalInput")
    out_t = nc.dram_tensor("out", [B, INNER, HLOC, W], F32, kind="ExternalOutput")

    ctx_ap = ctx_t.ap()
    x_ap = x_t.ap()
    out_ap = out_t.ap()

    with tile.TileContext(nc) as tc, ExitStack() as ctxs:
        ep = ctxs.enter_context

        const_pool = ep(tc.tile_pool(name="const", bufs=1))
        ctx_pool = ep(tc.tile_pool(name="ctxp", bufs=9))
        x_pool = ep(tc.tile_pool(name="xp", bufs=2))
        q_pool = ep(tc.tile_pool(name="qp", bufs=2))
        tmp_pool = ep(tc.tile_pool(name="tmpp", bufs=6))
        s8_pool = ep(tc.tile_pool(name="s8p", bufs=2))
        ebc_pool = ep(tc.tile_pool(name="ebcp", bufs=3))
        mv_pool = ep(tc.tile_pool(name="mvp", bufs=3))
        sm_pool = ep(tc.tile_pool(name="smp", bufs=2))
        y_pool = ep(tc.tile_pool(name="yp", bufs=2))

        kps_pool = ep(tc.tile_pool(name="kps", bufs=2, space="PSUM"))
        vps_pool = ep(tc.tile_pool(name="vps", bufs=2, space="PSUM"))
        eps_pool = ep(tc.tile_pool(name="eps", bufs=2, space="PSUM"))
        sps_pool = ep(tc.tile_pool(name="sps", bufs=2, space="PSUM"))

        # ---- constants ----
        wq_sb = const_pool.tile([128, 960], BF16, tag="wq")
        wk_sb = const_pool.tile([128, 960], BF16, tag="wk")
        wv_sb = const_pool.tile([128, 984], BF16, tag="wv")
        wo_sb = const_pool.tile([128, 960], BF16, tag="wo")
        sel_sb = const_pool.tile([128, 24], BF16, tag="sel")
        bsel_sb = const_pool.tile([8, 384], BF16, tag="bsel")
        rsel_sb = const_pool.tile([128, 384], F32R, tag="rsel")
        bout_sb = const_pool.tile([128, 3], F32, tag="bout")
        for sb, dr in ((wq_sb, wq_t), (wk_sb, wk_t), (wv_sb, wv_t),
                       (wo_sb, wo_t), (sel_sb, sel_t), (bsel_sb, bsel_t),
                       (bout_sb, bout_t)):
            nc.sync.dma_start(sb[:], dr.ap())
        nc.sync.dma_start(rsel_sb[64:72, :], rsel_t.ap())

        def phase_a(blk):
            """DMA in, q projection, k projection, k*q, selector-reduce,
            drain sim to sbuf. Returns (ctx_sb, s8_sb)."""
            ctx_sb = []
            for kc, (c0, csz) in enumerate(CHUNKS):
                t = ctx_pool.tile([128, D * P], BF16, tag="ctx")
                ksz = csz + 1 if kc == 2 else csz  # chunk2 + ones-channel
                nc.sync.dma_start(t[0:ksz, :], ctx_ap[blk, c0:c0 + ksz, :])
                ctx_sb.append(t)
            x_sb = x_pool.tile([128, 384], BF16, tag="x")
            for kc, (c0, csz) in enumerate(CHUNKS):
                nc.sync.dma_start(x_sb[0:csz, kc * P:kc * P + P],
                                  x_ap[blk, c0:c0 + csz, :])

            q_ps = kps_pool.tile([128, 512], F32, tag="kp")
            for mo, (o0, osz) in enumerate(CHUNKS):
                for kc, (c0, csz) in enumerate(CHUNKS):
                    nc.tensor.matmul(
                        q_ps[0:osz, mo * P:mo * P + P],
                        wq_sb[0:csz, kc * 320 + o0:kc * 320 + o0 + osz],
                        x_sb[0:csz, kc * P:kc * P + P],
                        start=(kc == 0), stop=(kc == 2),
                    )
            q_sb = q_pool.tile([128, 384], BF16, tag="q")
            for mo, (o0, osz) in enumerate(CHUNKS):
                nc.scalar.activation(q_sb[0:osz, mo * P:mo * P + P],
                                     q_ps[0:osz, mo * P:mo * P + P],
                                     mybir.ActivationFunctionType.Copy)

            # k free layout: (d, pix) d-major; 512 = 4 d x 128 pix
            s8_sb = s8_pool.tile([8, D * P], BF16, tag="s8")
            for nt in range(NT):
                tmp_ts = []
                for mo, (o0, osz) in enumerate(CHUNKS):
                    kp = kps_pool.tile([128, 512], F32, tag="kp")
                    for kc, (c0, csz) in enumerate(CHUNKS):
                        nc.tensor.matmul(
                            kp[0:osz, :],
                            wk_sb[0:csz, kc * 320 + o0:kc * 320 + o0 + osz],
                            ctx_sb[kc][0:csz, nt * 512:(nt + 1) * 512],
                            start=(kc == 0), stop=(kc == 2),
                        )
                    tmp_t = tmp_pool.tile([128, 512], BF16, tag="tmp")
                    qb = q_sb[0:osz, mo * P:mo * P + P].unsqueeze(1).to_broadcast(
                        (osz, 4, P))
                    nc.vector.tensor_mul(
                        tmp_t[0:osz, :].rearrange("c (a x) -> c a x", a=4),
                        kp[0:osz, :].rearrange("c (a x) -> c a x", a=4),
                        qb,
                    )
                    tmp_ts.append(tmp_t)
                sim_ps = sps_pool.tile([8, 512], F32, tag="sp")
                for mo, (o0, osz) in enumerate(CHUNKS):
                    nc.tensor.matmul(
                        sim_ps[0:8, :],
                        sel_sb[0:osz, mo * 8:mo * 8 + 8],
                        tmp_ts[mo][0:osz, :],
                        start=(mo == 0), stop=(mo == 2),
                    )
                # contiguous drain (sim stays d-major in s8)
                nc.scalar.activation(s8_sb[0:8, nt * 512:(nt + 1) * 512],
                                     sim_ps[0:8, :],
                                     mybir.ActivationFunctionType.Copy)
            return ctx_sb, s8_sb

        def phase_b(blk, ctx_sb, s8_sb):
            """Broadcast+exp, v projection, v*attn, d-reduce, normalize,
            output projection, DMA out."""
            b = blk // (PIX_B // P)
            p0 = (blk % (PIX_B // P)) * P
            hr = p0 // W
            nh = P // W

            ebc_sb = []
            for mo in range(3):
                bsz = VSZ[mo]
                t = ebc_pool.tile([128, D * P], BF16, tag="ebc")
                for nt in range(NT):
                    e_ps = eps_pool.tile([128, 512], F32, tag="ep")
                    nc.tensor.matmul(
                        e_ps[0:bsz, :],
                        bsel_sb[0:8, mo * 128:mo * 128 + bsz],
                        s8_sb[0:8, nt * 512:(nt + 1) * 512],
                    )
                    nc.scalar.activation(t[0:bsz, nt * 512:(nt + 1) * 512],
                                         e_ps[0:bsz, :],
                                         mybir.ActivationFunctionType.Exp)
                ebc_sb.append(t)

            mv_sb = []
            for mo in range(3):
                osz = VSZ[mo]
                o0 = CHUNKS[mo][0]
                t = mv_pool.tile([128, D * P], BF16, tag="mv")
                for nt in range(NT):
                    vp = vps_pool.tile([128, 512], F32, tag="vp")
                    for kc, (c0, csz) in enumerate(CHUNKS):
                        ksz = csz + 1 if kc == 2 else csz
                        nc.tensor.matmul(
                            vp[0:osz, :],
                            wv_sb[0:ksz, kc * 328 + o0:kc * 328 + o0 + osz],
                            ctx_sb[kc][0:ksz, nt * 512:(nt + 1) * 512],
                            start=(kc == 0), stop=(kc == 2),
                        )
                    nc.vector.tensor_mul(
                        t[0:osz, nt * 512:(nt + 1) * 512],
                        vp[0:osz, :],
                        ebc_sb[mo][0:osz, nt * 512:(nt + 1) * 512],
                    )
                mv_sb.append(t)

            ov_sb = sm_pool.tile([128, 384], F32, tag="ov")
            for mo in range(3):
                nc.vector.tensor_reduce(
                    ov_sb[0:VSZ[mo], mo * P:mo * P + P],
                    mv_sb[mo][0:VSZ[mo], :].rearrange("c (d x) -> c x d", d=D),
                    axis=mybir.AxisListType.X,
                    op=mybir.AluOpType.add,
                )
            r8_sb = sm_pool.tile([128, P], F32R, tag="r8")
            with nc.allow_low_precision(reason="f32r reciprocal feeding matmul"):
                nc.vector.reciprocal(r8_sb[64:72, :], ov_sb[64:72, 2 * P:3 * P])
            att_sb = sm_pool.tile([128, 384], BF16, tag="att")
            for mo, (o0, osz) in enumerate(CHUNKS):
                r_ps = eps_pool.tile([128, 512], F32, tag="ep")
                nc.tensor.matmul(
                    r_ps[0:osz, 0:P],
                    rsel_sb[64:72, mo * 128:mo * 128 + osz],
                    r8_sb[64:72, :],
                )
                nc.vector.tensor_mul(
                    att_sb[0:osz, mo * P:mo * P + P],
                    ov_sb[0:osz, mo * P:mo * P + P],
                    r_ps[0:osz, 0:P],
                )

            y_ps = vps_pool.tile([128, 512], F32, tag="vp")
            for mo, (o0, osz) in enumerate(CHUNKS):
                for kc, (c0, csz) in enumerate(CHUNKS):
                    nc.tensor.matmul(
                        y_ps[0:osz, mo * P:mo * P + P],
                        wo_sb[0:csz, kc * 320 + o0:kc * 320 + o0 + osz],
                        att_sb[0:csz, kc * P:kc * P + P],
                        start=(kc == 0), stop=(kc == 2),
                    )
            y_sb = y_pool.tile([128, 384], F32, tag="y")
            for mo, (o0, osz) in enumerate(CHUNKS):
                nc.vector.tensor_scalar_add(
                    y_sb[0:osz, mo * P:mo * P + P],
                    y_ps[0:osz, mo * P:mo * P + P],
                    bout_sb[0:osz, mo:mo + 1],
                )
            for mo, (o0, osz) in enumerate(CHUNKS):
                dst = out_ap[b, o0:o0 + osz, hr:hr + nh, :].rearrange(
                    "c h w -> c (h w)")
                nc.sync.dma_start(dst, y_sb[0:osz, mo * P:mo * P + P])

        # software pipeline, depth 2
        state = {}
        for blk in range(NBLK):
            state[blk] = phase_a(blk)
            if blk >= 1:
                phase_b(blk - 1, *state.pop(blk - 1))
        phase_b(NBLK - 1, *state.pop(NBLK - 1))

    nc.compile()
    return nc


_CACHED = {}


def _get_nc():
    if "nc" not in _CACHED:
        _CACHED["nc"] = build_nc()
    return _CACHED["nc"]


def make_core_inputs(x, context, wq, wk, wv, wout, bout):
    """Full inputs -> list of 8 per-core input dicts (host prep: shard,
    block, append ones-channel, cast to bf16)."""
    sel, bsel, rsel = make_constants()
    consts = {
        "wq_p": pack_weight_T(np.asarray(wq, np.float32) * SCALE),
        "wk_p": pack_weight_T(np.asarray(wk, np.float32)),
        "wv_p": pack_weight_T(np.asarray(wv, np.float32), ones_cols=True),
        "wo_p": pack_weight_T(np.asarray(wout, np.float32)),
        "sel_p": sel, "bsel_p": bsel, "rsel_p": rsel,
    }
    bout_p = np.zeros((128, 3), np.float32)
    for mo, (o0, osz) in enumerate(CHUNKS):
        bout_p[0:osz, mo] = np.asarray(bout, np.float32)[o0:o0 + osz]
    consts["bout_p"] = bout_p
    x = np.asarray(x, np.float32)
    context = np.asarray(context, np.float32)
    nbh = PIX_B // P  # 4
    in_maps = []
    for cid in range(NCORES):
        h0 = cid * HLOC
        cs = context[:, :, :, h0:h0 + HLOC, :]  # [B, C, D, HLOC, W]
        cs = cs.reshape(B, CIN, D, nbh, P).transpose(0, 3, 1, 2, 4)
        cs = cs.reshape(NBLK, CIN, D * P)
        panel = np.ones((NBLK, CIN + 1, D * P), NPBF)
        panel[:, 0:CIN, :] = cs.astype(NPBF)
        xs = x[:, :, h0:h0 + HLOC, :].reshape(B, CIN, nbh, P).transpose(0, 2, 1, 3)
        xs = np.ascontiguousarray(xs.reshape(NBLK, CIN, P), dtype=NPBF)
        m = dict(consts)
        m["ctx"] = panel
        m["x"] = xs
        in_maps.append(m)
    return in_maps


def kernel(x, context, wq, wk, wv, wout, bout):
    from concourse.bass_utils import run_bass_kernel_spmd

    nc = _get_nc()
    in_maps = make_core_inputs(x, context, wq, wk, wv, wout, bout)
    res = run_bass_kernel_spmd(nc, in_maps, list(range(NCORES)))
    shards = [res.results[c]["out"] for c in range(NCORES)]
    return np.concatenate(shards, axis=2).astype(np.float32)


if __name__ == "__main__":
    nc = build_nc()
    print("build + compile OK")



# revision 6
# speedup vs baseline: 1.1464x; 1.1464x over previous
"""Trainium2 Bass kernel for nn_DepthAttention (depth attention over d=32).

Reference computation (per pixel (b,h,w), all 1x1 convs):
  q = Wq x               [320]   (heads=8 x dh=40)
  k = Wk ctx[:, d]       [320, 32]
  v = Wv ctx[:, d]       [320, 32]
  sim[n,d] = sum_{c in head n} q[c] k[c,d] * scale
  attn = softmax_d(sim)
  o[c] = sum_d v[c,d] attn[head(c),d]
  y = Wout o + bout      [320]

Sharding: h (64) split across 8 cores -> 8 rows of h per core, no halo.
Per core: 1024 pixels in 8 blocks of P=128 (free layout d-major:
col = d_sub*128 + pixel, 8 nt-tiles of 512 cols each).

Key performance structure (vs the naive per-phase version):
  * k-proj and v-proj are merged into ONE 640-row output matmul set of
    5 m-tiles x 3 contraction passes (Sigma cols = 15*512/nt instead of
    18*512): T0/T1 = k slots, T2 = k slots 256:320 || v ch 0:64,
    T3/T4 = v ch 64:192/192:320.
  * k/q output channels are permuted (pi) so each of the 128 rows of the
    three k-chunks holds channels of a single head; the 320->8 head
    reduction (sel) contracts the three q*k product chunks directly.
  * PE emission is software-pipelined with skew 1: per nt iteration
    [sel(nt-1) | kvT0,T1(nt) | bcast(nt-1)x3 | kvT2..T4(nt)], so the
    sel->exp->bcast cross-engine chain hides under the kv matmuls and
    the tensor engine stays dense (p-state ramps to 2.4 GHz).
  * d-reduction of v*attn uses log2 tree adds on DVE (bf16 2x mode)
    instead of strided tensor_reduce.
  * Engine split: ACT = k-psum drains + exp + q/y drains; DVE = q*k
    products (bf16 2x), v*attn (direct PSUM), trees, recip, normalize.

PSUM budget (8 banks): t0..t4 (bufs=1 each) + ebc (bufs=2) + sel
(bufs=1); q-proj reuses tag ebc, wout reuses t3, recip-bcast reuses t4.
"""

import sys

sys.path.insert(0, "/opt/trn_rl_repo")

from contextlib import ExitStack  # noqa: E402

import ml_dtypes  # noqa: E402
import numpy as np  # noqa: E402

import concourse.bacc as bacc  # noqa: E402
import concourse.bass as bass  # noqa: E402
import concourse.mybir as mybir  # noqa: E402
import concourse.tile as tile  # noqa: E402

HEADS = 8
DH = 40
CIN = 320
INNER = HEADS * DH  # 320
D = 32
B = 2
H = 64
W = 64
NCORES = 8
HLOC = H // NCORES  # 8
PIX_B = HLOC * W  # 512
P = 128
NBLK = B * PIX_B // P  # 8
NT = (D * P) // 512  # 8
SCALE = DH ** -0.5

F32 = mybir.dt.float32
F32R = mybir.dt.float32r
BF16 = mybir.dt.bfloat16
NPBF = ml_dtypes.bfloat16

# slot chunks: q/k out rows, x/ctx contraction rows, y out rows
QCH = [(0, 128), (128, 128), (256, 64)]
# v channel groups: rows of (T2[64:128], T3, T4)
VCH = [(0, 64), (64, 128), (192, 128)]


def make_perm():
    """Slot -> original channel. Rows r<64 host 3 channels (slots r,
    r+128, r+256), rows 64..127 host 2 (slots r, r+128); all channels in
    one row belong to the same head: head_of_row = r//8 (r<64) else
    (r-64)//8."""
    perm = np.zeros(320, np.int64)
    for h in range(8):
        for j in range(8):
            r = 8 * h + j
            perm[r] = 40 * h + 3 * j
            perm[128 + r] = 40 * h + 3 * j + 1
            perm[256 + r] = 40 * h + 3 * j + 2
            r2 = 64 + 8 * h + j
            perm[r2] = 40 * h + 24 + 2 * j
            perm[128 + r2] = 40 * h + 24 + 2 * j + 1
    return perm


def head_of_row(r):
    return r // 8 if r < 64 else (r - 64) // 8


def pack_weights(wq, wk, wv, wout, bout):
    perm = make_perm()
    wqs = (np.asarray(wq, np.float32) * SCALE)[perm, :]  # [320 slots, 320 in]
    wks = np.asarray(wk, np.float32)[perm, :]
    wvv = np.asarray(wv, np.float32)

    wqp = np.zeros((128, 960), NPBF)
    for kc, (c0, csz) in enumerate(QCH):
        wqp[0:csz, kc * 320:kc * 320 + 320] = wqs[:, c0:c0 + csz].T

    # merged kv: 5 m-tiles x 3 contraction chunks, each [csz, 128]
    tile_srcs = [
        wks[0:128, :],
        wks[128:256, :],
        np.concatenate([wks[256:320, :], wvv[0:64, :]], axis=0),
        wvv[64:192, :],
        wvv[192:320, :],
    ]
    wkvp = np.zeros((128, 1920), NPBF)
    for t, src in enumerate(tile_srcs):
        for kc, (c0, csz) in enumerate(QCH):
            wkvp[0:csz, (t * 3 + kc) * 128:(t * 3 + kc + 1) * 128] = \
                src[:, c0:c0 + csz].T

    selw = np.zeros((128, 8), NPBF)
    for r in range(128):
        selw[r, head_of_row(r)] = 1.0

    bsel = np.zeros((8, 320), NPBF)
    rsel = np.zeros((8, 320), np.float32)
    for c in range(320):
        bsel[c // DH, c] = 1.0
        rsel[c // DH, c] = 1.0

    wop = np.zeros((128, 960), NPBF)
    wo = np.asarray(wout, np.float32)
    for kc, (v0, vsz) in enumerate(VCH):
        wop[0:vsz, kc * 320:kc * 320 + 320] = wo[:, v0:v0 + vsz].T

    boutp = np.zeros((128, 3), np.float32)
    for mo, (o0, osz) in enumerate(QCH):
        boutp[0:osz, mo] = np.asarray(bout, np.float32)[o0:o0 + osz]

    return {"wq_p": wqp, "wkv_p": wkvp, "sel_p": selw, "bsel_p": bsel,
            "rsel_p": rsel, "wo_p": wop, "bout_p": boutp}


def build_nc():
    nc = bacc.Bacc(
        "TRN2",
        target_bir_lowering=False,
        debug=False,
        enable_asserts=False,
        num_devices=NCORES,
    )

    ctx_t = nc.dram_tensor("ctx", [NBLK, CIN, D * P], BF16, kind="ExternalInput")
    x_t = nc.dram_tensor("x", [NBLK, CIN, P], BF16, kind="ExternalInput")
    wq_t = nc.dram_tensor("wq_p", [128, 960], BF16, kind="ExternalInput")
    wkv_t = nc.dram_tensor("wkv_p", [128, 1920], BF16, kind="ExternalInput")
    sel_t = nc.dram_tensor("sel_p", [128, 8], BF16, kind="ExternalInput")
    bsel_t = nc.dram_tensor("bsel_p", [8, 320], BF16, kind="ExternalInput")
    rsel_t = nc.dram_tensor("rsel_p", [8, 320], F32R, kind="ExternalInput")
    wo_t = nc.dram_tensor("wo_p", [128, 960], BF16, kind="ExternalInput")
    bout_t = nc.dram_tensor("bout_p", [128, 3], F32, kind="ExternalInput")
    out_t = nc.dram_tensor("out", [B, INNER, HLOC, W], F32, kind="ExternalOutput")

    ctx_ap = ctx_t.ap()
    x_ap = x_t.ap()
    out_ap = out_t.ap()
    AF = mybir.ActivationFunctionType

    with tile.TileContext(nc) as tc, ExitStack() as ctxs:
        ep = ctxs.enter_context
        cpool = ep(tc.tile_pool(name="const", bufs=1))
        dpool = ep(tc.tile_pool(name="data", bufs=2))
        pspool = ep(tc.tile_pool(name="ps", bufs=1, space="PSUM"))

        wq_sb = cpool.tile([128, 960], BF16, tag="wq")
        wkv_sb = cpool.tile([128, 1920], BF16, tag="wkv")
        sel_sb = cpool.tile([128, 8], BF16, tag="sel")
        bsel_sb = cpool.tile([8, 320], BF16, tag="bsel")
        rsel_sb = cpool.tile([8, 320], F32R, tag="rsel")
        wo_sb = cpool.tile([128, 960], BF16, tag="wo")
        bout_sb = cpool.tile([128, 3], F32, tag="bout")
        for sb, dr in ((wq_sb, wq_t), (wkv_sb, wkv_t), (wo_sb, wo_t),
                       (sel_sb, sel_t), (bout_sb, bout_t)):
            nc.sync.dma_start(sb[:], dr.ap())
        nc.sync.dma_start(bsel_sb[0:8, :], bsel_t.ap())
        nc.sync.dma_start(rsel_sb[0:8, :], rsel_t.ap())

        blkst = {}  # blk -> dict
        ntst = {}   # global nt -> dict

        def emit_dma(blk):
            st = blkst.setdefault(blk, {})
            ctx_sb = []
            for kc, (c0, csz) in enumerate(QCH):
                t = dpool.tile([128, D * P], BF16, tag=f"ctx{kc}", bufs=2,
                               name=f"ctx{kc}")
                nc.sync.dma_start(t[0:csz, :], ctx_ap[blk, c0:c0 + csz, :])
                ctx_sb.append(t)
            x_sb = dpool.tile([128, 384], BF16, tag="x", bufs=2, name="x_sb")
            for kc, (c0, csz) in enumerate(QCH):
                nc.sync.dma_start(x_sb[0:csz, kc * P:(kc + 1) * P],
                                  x_ap[blk, c0:c0 + csz, :])
            st["ctx"] = ctx_sb
            st["x"] = x_sb

        def emit_qproj(blk):
            st = blkst[blk]
            x_sb = st["x"]
            q_sb = dpool.tile([128, 384], BF16, tag="q", bufs=2, name="q_sb")
            for mo, (o0, osz) in enumerate(QCH):
                q_ps = pspool.tile([128, 512], F32, tag="ebc", bufs=2,
                                   name="q_ps")
                for kc, (c0, csz) in enumerate(QCH):
                    nc.tensor.matmul(
                        q_ps[0:osz, 0:P],
                        wq_sb[0:csz, kc * 320 + o0:kc * 320 + o0 + osz],
                        x_sb[0:csz, kc * P:(kc + 1) * P],
                        start=(kc == 0), stop=(kc == 2),
                    )
                nc.scalar.activation(q_sb[0:osz, mo * P:(mo + 1) * P],
                                     q_ps[0:osz, 0:P], AF.Copy)
            st["q"] = q_sb
            st["s8e"] = dpool.tile([8, D * P], BF16, tag="s8e", bufs=2,
                                   name="s8e")
            st["mv"] = [
                dpool.tile([128, D * P], BF16, tag=f"mv{i}", bufs=2,
                           name=f"mv{i}")
                for i in range(3)
            ]

        def sel_part(g):
            """Head-reduce the q*k products of iteration g (emitted at the
            tail of iteration g's PE stream) and exp-drain to s8e."""
            st = ntst[g]
            blk, nt = divmod(g, NT)
            sim_ps = pspool.tile([8, 512], F32, tag="t4", bufs=1,
                                 name="sim_ps")
            for kc, (c0, csz) in enumerate(QCH):
                nc.tensor.matmul(sim_ps[0:8, :], sel_sb[0:csz, :],
                                 st["prod"][kc][0:csz, :],
                                 start=(kc == 0), stop=(kc == 2))
            s8e = blkst[blk]["s8e"]
            nc.scalar.activation(s8e[0:8, nt * 512:(nt + 1) * 512],
                                 sim_ps[0:8, :], AF.Exp)

        def kv_part(g, tiles):
            blk, nt = divmod(g, NT)
            st = ntst.setdefault(g, {"kv": [None] * 5, "prod": [None] * 3,
                                     "vd": [None] * 3})
            ctx_sb = blkst[blk]["ctx"]
            q_sb = blkst[blk]["q"]
            for t in tiles:
                bufs = 2 if t == 2 else 1
                ps = pspool.tile([128, 512], F32, tag=f"t{t}", bufs=bufs,
                                 name=f"kv{t}")
                for kc, (c0, csz) in enumerate(QCH):
                    nc.tensor.matmul(
                        ps[0:128, :],
                        wkv_sb[0:csz, (t * 3 + kc) * 128:(t * 3 + kc + 1) * 128],
                        ctx_sb[kc][0:csz, nt * 512:(nt + 1) * 512],
                        start=(kc == 0), stop=(kc == 2),
                    )
                st["kv"][t] = ps
                if t <= 2:
                    # q*k product straight from PSUM (one PSUM operand)
                    rows = 128 if t <= 1 else 64
                    prod = dpool.tile([128, 512], BF16, tag=f"prod{t}",
                                      bufs=2, name=f"prod{t}")
                    qb = q_sb[0:rows, t * P:(t + 1) * P].unsqueeze(1).to_broadcast(
                        (rows, 4, P))
                    nc.vector.tensor_mul(
                        prod[0:rows, :].rearrange("c (a x) -> c a x", a=4),
                        ps[0:rows, :].rearrange("c (a x) -> c a x", a=4),
                        qb,
                    )
                    st["prod"][t] = prod
                if t >= 3:
                    # drain v rows to SBUF so the v*attn mul runs in bf16 2x
                    vsz = VCH[t - 2][1]
                    vd = dpool.tile([128, 512], BF16, tag=f"vd{t}", bufs=2,
                                    name=f"vd{t}")
                    nc.scalar.activation(vd[0:vsz, :], ps[0:vsz, :], AF.Copy)
                    st["vd"][t - 2] = vd

        def bcast_vmul(g):
            st = ntst[g]
            blk, nt = divmod(g, NT)
            s8e = blkst[blk]["s8e"]
            mvs = blkst[blk]["mv"]
            for i, (v0, vsz) in enumerate(VCH):
                ebc = pspool.tile([128, 512], F32, tag="ebc", bufs=2,
                                  name=f"ebc{i}")
                nc.tensor.matmul(ebc[0:vsz, :], bsel_sb[0:8, v0:v0 + vsz],
                                 s8e[0:8, nt * 512:(nt + 1) * 512])
                eb = dpool.tile([128, 512], BF16, tag=f"eb{i}", bufs=2,
                                name=f"eb{i}")
                nc.scalar.activation(eb[0:vsz, :], ebc[0:vsz, :], AF.Copy)
                if i == 0:
                    vp = st["kv"][2][64:128, :]
                else:
                    vp = st["vd"][i][0:vsz, :]
                nc.vector.tensor_mul(mvs[i][0:vsz, nt * 512:(nt + 1) * 512],
                                     vp, eb[0:vsz, :])

        def tree_reduce(eng, src, rows, name):
            """src [rows, 4096] (d-major) -> [rows, 128] f32 via 5 halving
            adds (bf16 intermediates)."""
            cur = src
            width = 2048
            lvl = 0
            while width >= 128:
                dt_out = BF16 if width > 128 else F32
                nxt = dpool.tile([128, width], dt_out, tag=f"L{lvl}", bufs=2,
                                 name=f"{name}_L{lvl}")
                eng.tensor_add(nxt[0:rows, :], cur[0:rows, 0:width],
                               cur[0:rows, width:2 * width])
                cur = nxt
                width //= 2
                lvl += 1
            return cur

        def emit_ep1(blk):
            st = blkst[blk]
            den = tree_reduce(nc.vector, st["s8e"], 8, f"den{blk}")
            rden = dpool.tile([8, P], F32R, tag="rden", bufs=2, name="rden")
            with nc.allow_low_precision(reason="f32r reciprocal feeding matmul"):
                nc.vector.reciprocal(rden[0:8, :], den[0:8, :])
            st["rden"] = rden
            st["ov"] = [None] * 3
            st["ov"][0] = tree_reduce(nc.gpsimd, st["mv"][0], 64, f"ov0_{blk}")

        def emit_ep2(blk):
            st = blkst[blk]
            st["ov"][1] = tree_reduce(nc.gpsimd, st["mv"][1], 128, f"ov1_{blk}")
            st["ov"][2] = tree_reduce(nc.gpsimd, st["mv"][2], 128, f"ov2_{blk}")
            att = dpool.tile([128, 384], BF16, tag="att", bufs=2, name="att")
            for i, (v0, vsz) in enumerate(VCH):
                rb = pspool.tile([128, 512], F32, tag="t4", bufs=1, name="rb")
                nc.tensor.matmul(rb[0:vsz, 0:P], rsel_sb[0:8, v0:v0 + vsz],
                                 st["rden"][0:8, :])
                nc.vector.tensor_mul(att[0:vsz, i * P:(i + 1) * P],
                                     st["ov"][i][0:vsz, :], rb[0:vsz, 0:P])
            st["att"] = att

        def emit_ep3(blk):
            st = blkst[blk]
            att = st["att"]
            y_ps = pspool.tile([128, 384], F32, tag="t3", bufs=1, name="y_ps")
            for mo, (o0, osz) in enumerate(QCH):
                for kc, (v0, vsz) in enumerate(VCH):
                    nc.tensor.matmul(
                        y_ps[0:osz, mo * P:mo * P + P],
                        wo_sb[0:vsz, kc * 320 + o0:kc * 320 + o0 + osz],
                        att[0:vsz, kc * P:(kc + 1) * P],
                        start=(kc == 0), stop=(kc == 2),
                    )
            y_sb = dpool.tile([128, 384], F32, tag="y", bufs=2, name="y_sb")
            for mo, (o0, osz) in enumerate(QCH):
                nc.vector.tensor_scalar_add(y_sb[0:osz, mo * P:mo * P + P],
                                            y_ps[0:osz, mo * P:mo * P + P],
                                            bout_sb[0:osz, mo:mo + 1])
            b = blk // (PIX_B // P)
            p0 = (blk % (PIX_B // P)) * P
            hr = p0 // W
            nh = P // W
            for mo, (o0, osz) in enumerate(QCH):
                dst = out_ap[b, o0:o0 + osz, hr:hr + nh, :].rearrange(
                    "c h w -> c (h w)")
                nc.sync.dma_start(dst, y_sb[0:osz, mo * P:mo * P + P])

        emit_dma(0)
        emit_qproj(0)
        TOT = NBLK * NT
        for g in range(TOT + 1):
            blk, nt = divmod(g, NT)
            if g < TOT:
                kv_part(g, [0, 1])
            if g >= 1:
                bcast_vmul(g - 1)
            if g < TOT:
                kv_part(g, [2, 3, 4])
                sel_part(g)
                if nt == 4 and blk + 1 < NBLK:
                    emit_dma(blk + 1)
                if nt == 6 and blk + 1 < NBLK:
                    emit_qproj(blk + 1)
                if nt == 1 and blk >= 1:
                    emit_ep1(blk - 1)
                if nt == 2 and blk >= 1:
                    emit_ep2(blk - 1)
                if nt == 3 and blk >= 1:
                    emit_ep3(blk - 1)
        emit_ep1(NBLK - 1)
        emit_ep2(NBLK - 1)
        emit_ep3(NBLK - 1)

    nc.compile()
    return nc


_CACHED = {}


def _get_nc():
    if "nc" not in _CACHED:
        _CACHED["nc"] = build_nc()
    return _CACHED["nc"]


def make_core_inputs(x, context, wq, wk, wv, wout, bout):
    """Full inputs -> list of 8 per-core input dicts (host prep: shard,
    block, cast to bf16, pack weights)."""
    consts = pack_weights(wq, wk, wv, wout, bout)
    x = np.asarray(x, np.float32)
    context = np.asarray(context, np.float32)
    nbh = PIX_B // P  # 4
    in_maps = []
    for cid in range(NCORES):
        h0 = cid * HLOC
        cs = context[:, :, :, h0:h0 + HLOC, :]  # [B, C, D, HLOC, W]
        cs = cs.reshape(B, CIN, D, nbh, P).transpose(0, 3, 1, 2, 4)
        cs = np.ascontiguousarray(cs.reshape(NBLK, CIN, D * P), dtype=NPBF)
        xs = x[:, :, h0:h0 + HLOC, :].reshape(B, CIN, nbh, P).transpose(0, 2, 1, 3)
        xs = np.ascontiguousarray(xs.reshape(NBLK, CIN, P), dtype=NPBF)
        m = dict(consts)
        m["ctx"] = cs
        m["x"] = xs
        in_maps.append(m)
    return in_maps


def kernel(x, context, wq, wk, wv, wout, bout):
    from concourse.bass_utils import run_bass_kernel_spmd

    nc = _get_nc()
    in_maps = make_core_inputs(x, context, wq, wk, wv, wout, bout)
    res = run_bass_kernel_spmd(nc, in_maps, list(range(NCORES)))
    shards = [res.results[c]["out"] for c in range(NCORES)]
    return np.concatenate(shards, axis=2).astype(np.float32)


if __name__ == "__main__":
    nc = build_nc()
    print("build + compile OK")
